# revision 5
# baseline (speedup 1.0000x reference)
"""GCN (4-layer) + mean-pool + linear head on 8 Trainium2 cores.

Strategy: shard destination nodes across 8 cores (load-balanced by degree),
aggregate-before-transform (S(HW) == (SH)W), so edge gathers happen at the
layer *input* width (5/32/64/128 instead of 32/64/128/256).

Per layer, per core:
  gather   h[src] rows from the replicated node-feature table (indirect DMA)
  scatter  psum[d_in, 32dst] += gathered_chunk[128e, d_in].T @ onehot_norm[128e, 32]
  evac     psum -> aggT (feature-major SBUF [d_in(+1), 6272])
  transform per 128-node subtile: psum[128n, d_out] = aggT_sub[d_in+1,128].T @ W'
           (ones-row in aggT folds the bias), relu -> node-major bf16
  allgather the [6272, d_out] slice -> full table [50176, d_out] for next layer
Layer 4 keeps h4 in SBUF; pooling via matmul against per-graph one-hots,
AllReduce [256, 64], replicated fp32 head + log_softmax on every core.
"""

import sys

for _p in ("/opt/trn_rl_repo", "/opt/pypackages"):
    if _p not in sys.path:
        sys.path.insert(0, _p)

from contextlib import ExitStack

import numpy as np
import ml_dtypes

import concourse.bass as bass
import concourse.tile as tile
from concourse import mybir
from concourse.bacc import Bacc
from concourse.masks import make_identity

BF16 = ml_dtypes.bfloat16

N = 50000      # nodes
E = 800000     # edges (without self loops)
G = 64         # graphs
C = 8          # cores
P = 128
GS = 32        # dst-group size (psum scatter column block)
NPC = 6272     # padded nodes per core  (= 196*32 = 49*128)
NT = NPC * C   # 50176 padded total
NSLOT = NPC // GS   # 196 dst-groups per core
NGRP = NSLOT * C    # 1568 groups total
NSUB = NPC // P     # 49 128-node subtiles per core
NCLS = 10

DIMS_IN = [8, 32, 64, 128]    # scatter/gather width per layer (L1 padded 5->8)
DIMS_OUT = [32, 64, 128, 256]


# ---------------------------------------------------------------- host side


def _preprocess(x, edge_index, batch):
    src = np.concatenate([edge_index[0].astype(np.int64), np.arange(N, dtype=np.int64)])
    dst = np.concatenate([edge_index[1].astype(np.int64), np.arange(N, dtype=np.int64)])
    M = src.shape[0]

    deg = np.bincount(dst, minlength=N).astype(np.float32)
    dinv = 1.0 / np.sqrt(deg)
    norm = (dinv[src] * dinv[dst]).astype(np.float32)

    # deal nodes (sorted by degree desc) round-robin into NGRP groups of <=32
    order = np.argsort(-deg, kind="stable")
    grp = np.empty(N, dtype=np.int64)
    rank = np.empty(N, dtype=np.int64)
    pos = np.arange(N)
    grp[order] = pos % NGRP
    rank[order] = pos // NGRP

    gcnt = np.bincount(grp[dst], minlength=NGRP)  # incoming edges per group
    # deal groups (sorted by edge count desc) into 8 cores x 196 slots so the
    # 8 groups sharing a slot have near-equal counts (SPMD chunk uniformity)
    gorder = np.argsort(-gcnt, kind="stable")
    core_of_grp = np.empty(NGRP, dtype=np.int64)
    slot_of_grp = np.empty(NGRP, dtype=np.int64)
    gpos = np.arange(NGRP)
    core_of_grp[gorder] = gpos % C
    slot_of_grp[gorder] = gpos // C

    cnt_cs = np.zeros((C, NSLOT), dtype=np.int64)
    cnt_cs[core_of_grp, slot_of_grp] = gcnt
    chunks_per_slot = np.maximum(1, -(-cnt_cs.max(axis=0) // P)).astype(np.int64)
    slot_start = np.zeros(NSLOT, dtype=np.int64)
    slot_start[1:] = np.cumsum(chunks_per_slot)[:-1]
    NCH = int(chunks_per_slot.sum())

    new_of_old = core_of_grp[grp] * NPC + slot_of_grp[grp] * GS + rank

    e_src = new_of_old[src]
    e_dst = new_of_old[dst]
    e_core = e_dst // NPC
    e_slot = (e_dst % NPC) // GS
    e_loc = e_dst % GS

    ekey = e_core * NSLOT + e_slot
    eord = np.argsort(ekey, kind="stable")
    skey = ekey[eord]
    cnts = np.bincount(ekey, minlength=C * NSLOT)
    starts = np.zeros(C * NSLOT, dtype=np.int64)
    starts[1:] = np.cumsum(cnts)[:-1]
    k = np.arange(M) - starts[skey]
    lane_p = k % P
    ch_glob = slot_start[skey % NSLOT] + k // P
    core_s = skey // NSLOT

    srcidx = np.zeros((C, P, NCH), dtype=np.int32)
    srcidx[core_s, lane_p, ch_glob] = e_src[eord].astype(np.int32)

    normhot = np.zeros((C, P, NCH, GS), dtype=np.float32)
    normhot[core_s, lane_p, ch_glob, e_loc[eord]] = norm[eord]

    xsrc = np.zeros((C, P, NCH, 8), dtype=np.float32)
    xsrc[core_s, lane_p, ch_glob, 0:5] = x[src[eord]]

    ghot = np.zeros((C, P, NSUB, G), dtype=np.float32)
    nn = new_of_old
    ghot[nn // NPC, nn % P, (nn % NPC) // P, batch.astype(np.int64)] = 1.0

    counts = np.bincount(batch.astype(np.int64), minlength=G).astype(np.float32)
    cinv = (1.0 / np.maximum(counts, 1.0)).reshape(1, G).astype(np.float32)

    meta = dict(NCH=NCH, chunks_per_slot=chunks_per_slot, slot_start=slot_start)
    arrays = dict(srcidx=srcidx, normhot=normhot, xsrc=xsrc, ghot=ghot, cinv=cinv)
    return meta, arrays


def _pack_weights(W1, b1, W2, b2, W3, b3, W4, b4, fcW, fcb):
    wp1 = np.zeros((33, 32), dtype=np.float32)
    wp1[0:5] = W1
    wp1[32] = b1
    wp2 = np.zeros((33, 64), dtype=np.float32)
    wp2[0:32] = W2
    wp2[32] = b2
    wp3 = np.zeros((65, 128), dtype=np.float32)
    wp3[0:64] = W3
    wp3[64] = b3
    return dict(
        wp1=wp1.astype(BF16),
        wp2=wp2.astype(BF16),
        wp3=wp3.astype(BF16),
        w4=W4.astype(BF16),
        b4b=b4.reshape(1, 256).astype(BF16),
        fcw=fcW.astype(np.float32),
        fcbt=fcb.reshape(NCLS, 1).astype(np.float32),
    )


def build_in_maps(inputs):
    meta, arr = _preprocess(
        np.asarray(inputs["x"], dtype=np.float32),
        np.asarray(inputs["edge_index"]),
        np.asarray(inputs["batch"]),
    )
    w = _pack_weights(
        *(np.asarray(inputs[k], dtype=np.float32) for k in (
            "W1", "b1", "W2", "b2", "W3", "b3", "W4", "b4", "fcW", "fcb"))
    )
    NCH = meta["NCH"]
    in_maps = []
    for c in range(C):
        m = dict(w)
        m["srcidx"] = arr["srcidx"][c]
        m["normhot"] = arr["normhot"][c].reshape(P, NCH * GS).astype(BF16)
        m["xsrc"] = arr["xsrc"][c].reshape(P, NCH * 8).astype(BF16)
        m["ghot"] = arr["ghot"][c].reshape(P, NSUB * G).astype(BF16)
        m["cinv"] = arr["cinv"]
        in_maps.append(m)
    return meta, in_maps


# -------------------------------------------------------------- device side


def build_program(meta, finalize=True):
    NCH = meta["NCH"]
    chunks_per_slot = meta["chunks_per_slot"]
    slot_start = meta["slot_start"]

    f32 = mybir.dt.float32
    bf16 = mybir.dt.bfloat16
    i32 = mybir.dt.int32
    AF = mybir.ActivationFunctionType
    OP = mybir.AluOpType
    groups = [list(range(C))]

    nc = Bacc("TRN2", target_bir_lowering=False, debug=False, num_devices=C)

    xsrc_d = nc.dram_tensor("xsrc", [P, NCH * 8], bf16, kind="ExternalInput")
    srcidx_d = nc.dram_tensor("srcidx", [P, NCH], i32, kind="ExternalInput")
    normhot_d = nc.dram_tensor("normhot", [P, NCH * GS], bf16, kind="ExternalInput")
    ghot_d = nc.dram_tensor("ghot", [P, NSUB * G], bf16, kind="ExternalInput")
    wp1_d = nc.dram_tensor("wp1", [33, 32], bf16, kind="ExternalInput")
    wp2_d = nc.dram_tensor("wp2", [33, 64], bf16, kind="ExternalInput")
    wp3_d = nc.dram_tensor("wp3", [65, 128], bf16, kind="ExternalInput")
    w4_d = nc.dram_tensor("w4", [128, 256], bf16, kind="ExternalInput")
    b4b_d = nc.dram_tensor("b4b", [1, 256], bf16, kind="ExternalInput")
    fcw_d = nc.dram_tensor("fcw", [256, NCLS], f32, kind="ExternalInput")
    fcbt_d = nc.dram_tensor("fcbt", [NCLS, 1], f32, kind="ExternalInput")
    cinv_d = nc.dram_tensor("cinv", [1, G], f32, kind="ExternalInput")
    out_d = nc.dram_tensor("out", [G, NCLS], f32, kind="ExternalOutput")

    with tile.TileContext(nc) as tc, ExitStack() as ctx:
        dram = ctx.enter_context(tc.tile_pool(name="dram", bufs=1, space="DRAM"))
        const = ctx.enter_context(tc.tile_pool(name="const", bufs=1))
        work = ctx.enter_context(tc.tile_pool(name="work", bufs=1))
        pp = ctx.enter_context(tc.tile_pool(name="ps", bufs=1, space="PSUM"))

        tin = [None] * 4
        tg = [None] * 4
        for li, d_out in enumerate(DIMS_OUT[:3]):
            tin[li] = dram.tile([NPC, d_out], bf16, tag=f"t{li}in", name=f"t{li}in")
            tg[li] = dram.tile([NT, d_out], bf16, tag=f"t{li}g", name=f"t{li}g",
                               addr_space="Shared")
        plin = dram.tile([256, G], f32, tag="plin", name="plin")
        plg = dram.tile([256, G], f32, tag="plg", name="plg", addr_space="Shared")

        # ---- constant loads
        def load(name, dten, shape, dtype):
            t = const.tile(shape, dtype, tag=name, name=name)
            nc.sync.dma_start(out=t[:], in_=dten[:])
            return t

        normhot_sb = load("normhot_sb", normhot_d, [P, NCH * GS], bf16)
        srcidx_sb = load("srcidx_sb", srcidx_d, [P, NCH], i32)
        xsrc_sb = load("xsrc_sb", xsrc_d, [P, NCH * 8], bf16)
        ghot_sb = load("ghot_sb", ghot_d, [P, NSUB * G], bf16)
        wp1_sb = load("wp1_sb", wp1_d, [33, 32], bf16)
        wp2_sb = load("wp2_sb", wp2_d, [33, 64], bf16)
        wp3_sb = load("wp3_sb", wp3_d, [65, 128], bf16)
        w4_sb = load("w4_sb", w4_d, [128, 256], bf16)
        b4b_sb = load("b4b_sb", b4b_d, [1, 256], bf16)
        fcb_sb = load("fcb_sb", fcbt_d, [NCLS, 1], f32)
        cinv_sb = load("cinv_sb", cinv_d, [1, G], f32)
        fcw_sb = const.tile([P, 2, NCLS], f32, tag="fcw_sb", name="fcw_sb")
        nc.sync.dma_start(out=fcw_sb[:, 0, :], in_=fcw_d[0:128, :])
        nc.sync.dma_start(out=fcw_sb[:, 1, :], in_=fcw_d[128:256, :])

        ones1_sb = const.tile([1, P], bf16, tag="ones1", name="ones1")
        nc.vector.memset(ones1_sb[:], 1.0)
        ones1f_sb = const.tile([1, P], f32, tag="ones1f", name="ones1f")
        nc.vector.memset(ones1f_sb[:], 1.0)
        ident_sb = const.tile([P, P], f32, tag="ident", name="ident")
        make_identity(nc, ident_sb[:])

        h4nm = const.tile([P, NSUB * 256], bf16, tag="h4nm", name="h4nm")

        # per-chunk slot id and first/last flags
        slot_of_chunk = np.repeat(np.arange(NSLOT), chunks_per_slot)
        is_first = np.zeros(NCH, dtype=bool)
        is_last = np.zeros(NCH, dtype=bool)
        is_first[slot_start] = True
        is_last[slot_start + chunks_per_slot - 1] = True

        def scatter_layer(li, d_in, lhsT_of_chunk, prefetch):
            """Accumulate aggT[d_in, NPC] for this layer; returns aggT tile."""
            k_rows = [33, 33, 65, 128][li]
            aggT = work.tile([k_rows, NPC], bf16, tag="aggT", bufs=2,
                             name=f"aggT{li}")
            if li == 0:
                # rows 5..31 must be exactly 0 (wp1 rows are 0 there, but
                # garbage*0 could be NaN); ones row sits at legal partition 32
                nc.vector.memset(aggT[:, :], 0.0)
                nc.vector.memset(aggT[32:33, :], 1.0)
            elif li < 3:
                nc.vector.memset(aggT[d_in:d_in + 1, :], 1.0)
            ps = None
            if prefetch is not None:
                prefetch(0)
            for ch in range(NCH):
                j = slot_of_chunk[ch]
                if is_first[ch] and j % 16 == 0:
                    ps = pp.tile([P, 512], f32, tag="ps_s", bufs=2,
                                 name=f"ps_s_{li}_{j}")
                if prefetch is not None and is_first[ch] and (j % 16 == 8):
                    # issue next gather batches spread through the block
                    pass
                col = (j % 16) * GS
                nc.tensor.matmul(
                    out=ps[:d_in, col:col + GS],
                    lhsT=lhsT_of_chunk(ch),
                    rhs=normhot_sb[:, ch * GS:(ch + 1) * GS],
                    start=is_first[ch],
                    stop=is_last[ch],
                )
                if is_last[ch] and (j % 16 == 15 or j == NSLOT - 1):
                    ncols = (j % 16 + 1) * GS
                    j0 = (j // 16) * 16
                    nc.scalar.copy(
                        out=aggT[:d_in, j0 * GS:j0 * GS + ncols],
                        in_=ps[:d_in, :ncols],
                    )
            return aggT

        def gathered_layer(li, d_in, table):
            """Indirect-gather h[src] in small batches; returns (lhsT_of_chunk, prefetch).

            Safety envelope (measured on HW): indirect SWDGE DMAs corrupt the
            gathered data once an instruction carries more than 256
            descriptors (descriptor staging wraps under drain backlog and
            clobbers in-flight descriptors), so batches are fixed at
            NB=2 chunks = 256 gathered rows per instruction.
            """
            NB = 2
            GBUFS = 2
            tiles = {}

            def prefetch(b):
                b0 = b * NB
                if b0 >= NCH or b in tiles:
                    return
                nb = min(NB, NCH - b0)
                # constant tile size across layers: one tag must always alias
                # same-size slots
                gt = work.tile([P, NB * 128], bf16, tag="gath", bufs=GBUFS,
                               name=f"gath_{li}_{b}")
                nc.gpsimd.indirect_dma_start(
                    out=gt[:, :nb * d_in],
                    out_offset=None,
                    in_=table[:],
                    in_offset=bass.IndirectOffsetOnAxis(
                        ap=srcidx_sb[:, b0:b0 + nb], axis=0),
                )
                tiles[b] = gt

            def lhsT_of_chunk(ch):
                b, o = divmod(ch, NB)
                prefetch(b + 1)
                return tiles[b][:, o * d_in:(o + 1) * d_in]

            return lhsT_of_chunk, prefetch

        def transform_layer(li, aggT, k_rows, w_sb, d_out):
            """Per 128-node subtile: relu(aggT_sub.T @ W'); emit node-major."""
            for t in range(NSUB):
                pst = pp.tile([P, 256], f32, tag="ps_t", bufs=2,
                              name=f"ps_t_{li}_{t}")
                nc.tensor.matmul(
                    out=pst[:, :d_out],
                    lhsT=aggT[:k_rows, t * P:(t + 1) * P],
                    rhs=w_sb[:k_rows, :d_out],
                    start=True,
                    stop=(li < 3),
                )
                if li == 3:
                    nc.tensor.matmul(
                        out=pst[:, :d_out],
                        lhsT=ones1_sb[:1, :P],
                        rhs=b4b_sb[:1, :256],
                        start=False,
                        stop=True,
                    )
                    nc.scalar.activation(
                        out=h4nm[:, t * 256:(t + 1) * 256],
                        in_=pst[:, :256], func=AF.Relu)
                else:
                    hb = work.tile([P, 256], bf16, tag="hbuf", bufs=3,
                                   name=f"hbuf_{li}_{t}")
                    nc.scalar.activation(
                        out=hb[:, :d_out], in_=pst[:, :d_out], func=AF.Relu)
                    nc.sync.dma_start(
                        out=tin[li][t * P:(t + 1) * P, :], in_=hb[:, :d_out])

        # ---- layer 1 (sources preloaded on host)
        aggT = scatter_layer(0, 8, lambda ch: xsrc_sb[:, ch * 8:(ch + 1) * 8], None)
        transform_layer(0, aggT, 33, wp1_sb, 32)
        nc.gpsimd.collective_compute(
            "AllGather", mybir.AluOpType.bypass, replica_groups=groups,
            ins=[tin[0].opt()], outs=[tg[0].opt()])

        # ---- layers 2..4
        for li, (d_in, d_out, w_sb) in enumerate(
                [(32, 64, wp2_sb), (64, 128, wp3_sb), (128, 256, w4_sb)], start=1):
            lhsT_of_chunk, prefetch = gathered_layer(li, d_in, tg[li - 1])
            aggT = scatter_layer(li, d_in, lhsT_of_chunk, prefetch)
            k_rows = d_in + 1 if li < 3 else d_in
            transform_layer(li, aggT, k_rows, w_sb, d_out)
            if li < 3:
                nc.gpsimd.collective_compute(
                    "AllGather", mybir.AluOpType.bypass, replica_groups=groups,
                    ins=[tin[li].opt()], outs=[tg[li].opt()])

        # ---- pooling: pooled.T[256, 64] = sum_t h4nm_sub.T @ ghot_sub
        psAB = pp.tile([P, 2 * G], f32, tag="psAB", name="psAB")
        poolsb = work.tile([P, 2 * G], f32, tag="poolsb", name="poolsb")
        for h in range(2):  # one accumulation group per psum bank at a time
            pst = psAB[:, h * G:(h + 1) * G]
            for t in range(NSUB):
                nc.tensor.matmul(
                    out=pst[:, :G],
                    lhsT=h4nm[:, t * 256 + h * P:t * 256 + h * P + P],
                    rhs=ghot_sb[:, t * G:(t + 1) * G],
                    start=(t == 0),
                    stop=(t == NSUB - 1),
                )
            nc.scalar.copy(out=poolsb[:, h * G:(h + 1) * G], in_=pst[:, :G])
        nc.sync.dma_start(out=plin[0:128, :], in_=poolsb[:, 0:G])
        nc.sync.dma_start(out=plin[128:256, :], in_=poolsb[:, G:2 * G])
        nc.gpsimd.collective_compute(
            "AllReduce", mybir.AluOpType.add, replica_groups=groups,
            ins=[plin.opt()], outs=[plg.opt()])

        # ---- head (replicated on every core)
        pool2 = work.tile([P, 2, G], f32, tag="pool2", name="pool2")
        nc.sync.dma_start(out=pool2[:, 0, :], in_=plg[0:128, :])
        nc.sync.dma_start(out=pool2[:, 1, :], in_=plg[128:256, :])
        psHead = pp.tile([P, 512], f32, tag="psHead", name="psHead")
        psc = psHead[:, 128:128 + G]
        nc.tensor.matmul(out=psc[:, :G], lhsT=ones1f_sb[:1, :P],
                         rhs=cinv_sb[:1, :G], start=True, stop=True)
        for h in range(2):
            nc.vector.tensor_tensor(
                out=pool2[:, h, :], in0=pool2[:, h, :],
                in1=psc[:, :G], op=OP.mult)
        psh = psHead[:, 0:G]
        nc.tensor.matmul(out=psh[:NCLS, :G], lhsT=fcw_sb[:, 0, :],
                         rhs=pool2[:, 0, :], start=True, stop=False)
        nc.tensor.matmul(out=psh[:NCLS, :G], lhsT=fcw_sb[:, 1, :],
                         rhs=pool2[:, 1, :], start=False, stop=True)
        lt = work.tile([NCLS, G], f32, tag="lt", name="lt")
        nc.scalar.activation(out=lt[:NCLS, :G], in_=psh[:NCLS, :G],
                             func=AF.Identity, bias=fcb_sb[:NCLS, :1], scale=1.0)
        pstr = psHead[:, 192:192 + NCLS]
        nc.tensor.transpose(out=pstr[:G, :NCLS], in_=lt[:NCLS, :G],
                            identity=ident_sb[:NCLS, :NCLS])
        l2 = work.tile([G, NCLS], f32, tag="l2", name="l2")
        nc.scalar.copy(out=l2[:, :], in_=pstr[:G, :NCLS])

        mx = work.tile([G, 1], f32, tag="mx", name="mx")
        nc.vector.tensor_reduce(out=mx[:, :], in_=l2[:, :],
                                axis=mybir.AxisListType.X, op=OP.max)
        l2m = work.tile([G, NCLS], f32, tag="l2m", name="l2m")
        nc.vector.tensor_scalar_sub(out=l2m[:, :], in0=l2[:, :], scalar1=mx[:, :1])
        ex = work.tile([G, NCLS], f32, tag="ex", name="ex")
        nc.scalar.activation(out=ex[:, :], in_=l2m[:, :], func=AF.Exp)
        sm = work.tile([G, 1], f32, tag="sm", name="sm")
        nc.vector.tensor_reduce(out=sm[:, :], in_=ex[:, :],
                                axis=mybir.AxisListType.X, op=OP.add)
        lsm = work.tile([G, 1], f32, tag="lsm", name="lsm")
        nc.scalar.activation(out=lsm[:, :], in_=sm[:, :], func=AF.Ln)
        res = work.tile([G, NCLS], f32, tag="res", name="res")
        nc.vector.tensor_scalar_sub(out=res[:, :], in0=l2m[:, :], scalar1=lsm[:, :1])
        nc.sync.dma_start(out=out_d[:], in_=res[:, :])

    if finalize:
        nc.finalize()
    return nc


# ------------------------------------------------------------------- entry


def kernel(**inputs):
    from concourse.bass_utils import run_bass_kernel_spmd

    meta, in_maps = build_in_maps(inputs)
    nc = build_program(meta)
    r = run_bass_kernel_spmd(nc, in_maps, list(range(C)))
    return np.asarray(r.results[0]["out"], dtype=np.float32)


if __name__ == "__main__":
    rng = np.random.default_rng(0)
    demo = {
        "x": rng.standard_normal((N, 5), dtype=np.float32),
        "edge_index": rng.integers(0, N, (2, E)).astype(np.int64),
        "batch": np.sort(rng.integers(0, G, N)).astype(np.int64),
        "W1": rng.standard_normal((5, 32), dtype=np.float32) * 0.1,
        "b1": np.zeros(32, np.float32),
        "W2": rng.standard_normal((32, 64), dtype=np.float32) * 0.1,
        "b2": np.zeros(64, np.float32),
        "W3": rng.standard_normal((64, 128), dtype=np.float32) * 0.1,
        "b3": np.zeros(128, np.float32),
        "W4": rng.standard_normal((128, 256), dtype=np.float32) * 0.1,
        "b4": np.zeros(256, np.float32),
        "fcW": rng.standard_normal((256, 10), dtype=np.float32) * 0.1,
        "fcb": np.zeros(10, np.float32),
    }
    print(kernel(**demo))



# revision 8
# speedup vs baseline: 1.3210x; 1.3210x over previous
"""GCN (4-layer) + mean-pool + linear head on 8 Trainium2 cores.

Strategy: shard destination nodes across 8 cores (load-balanced by degree),
aggregate-before-transform (S(HW) == (SH)W), so edge gathers happen at the
layer *input* width (5/32/64/128 instead of 32/64/128/256).

Per layer, per core:
  gather   h[src] rows from the replicated node-feature table (indirect DMA)
  scatter  psum[d_in, 32dst] += gathered_chunk[128e, d_in].T @ onehot_norm[128e, 32]
  evac     psum -> aggT (feature-major SBUF [d_in(+1), 6272])
  transform per 128-node subtile: psum[128n, d_out] = aggT_sub[d_in+1,128].T @ W'
           (ones-row in aggT folds the bias), relu -> node-major bf16
  allgather the [6272, d_out] slice -> full table [50176, d_out] for next layer
Layer 4 keeps h4 in SBUF; pooling via matmul against per-graph one-hots,
AllReduce [256, 64], replicated fp32 head + log_softmax on every core.
"""

import sys

for _p in ("/opt/trn_rl_repo", "/opt/pypackages"):
    if _p not in sys.path:
        sys.path.insert(0, _p)

from contextlib import ExitStack

import numpy as np
import ml_dtypes

import concourse.bass as bass
import concourse.tile as tile
from concourse import mybir
from concourse.bacc import Bacc
from concourse.masks import make_identity

BF16 = ml_dtypes.bfloat16

N = 50000      # nodes
E = 800000     # edges (without self loops)
G = 64         # graphs
C = 8          # cores
P = 128
GS = 32        # dst-group size (psum scatter column block)
NPC = 6272     # padded nodes per core  (= 196*32 = 49*128)
NT = NPC * C   # 50176 padded total
NSLOT = NPC // GS   # 196 dst-groups per core
NGRP = NSLOT * C    # 1568 groups total
NSUB = NPC // P     # 49 128-node subtiles per core
NCLS = 10

DIMS_IN = [8, 32, 64, 128]    # scatter/gather width per layer (L1 padded 5->8)
DIMS_OUT = [32, 64, 128, 256]


# ---------------------------------------------------------------- host side


def _preprocess(x, edge_index, batch):
    src = np.concatenate([edge_index[0].astype(np.int64), np.arange(N, dtype=np.int64)])
    dst = np.concatenate([edge_index[1].astype(np.int64), np.arange(N, dtype=np.int64)])
    M = src.shape[0]

    deg = np.bincount(dst, minlength=N).astype(np.float32)
    dinv = 1.0 / np.sqrt(deg)
    norm = (dinv[src] * dinv[dst]).astype(np.float32)

    # deal nodes (sorted by degree desc) round-robin into NGRP groups of <=32
    order = np.argsort(-deg, kind="stable")
    grp = np.empty(N, dtype=np.int64)
    rank = np.empty(N, dtype=np.int64)
    pos = np.arange(N)
    grp[order] = pos % NGRP
    rank[order] = pos // NGRP

    gcnt = np.bincount(grp[dst], minlength=NGRP)  # incoming edges per group
    # deal groups (sorted by edge count desc) into 8 cores x 196 slots so the
    # 8 groups sharing a slot have near-equal counts (SPMD chunk uniformity)
    gorder = np.argsort(-gcnt, kind="stable")
    core_of_grp = np.empty(NGRP, dtype=np.int64)
    slot_of_grp = np.empty(NGRP, dtype=np.int64)
    gpos = np.arange(NGRP)
    core_of_grp[gorder] = gpos % C
    slot_of_grp[gorder] = gpos // C

    cnt_cs = np.zeros((C, NSLOT), dtype=np.int64)
    cnt_cs[core_of_grp, slot_of_grp] = gcnt
    chunks_per_slot = np.maximum(1, -(-cnt_cs.max(axis=0) // P)).astype(np.int64)
    slot_start = np.zeros(NSLOT, dtype=np.int64)
    slot_start[1:] = np.cumsum(chunks_per_slot)[:-1]
    NCH = int(chunks_per_slot.sum())

    new_of_old = core_of_grp[grp] * NPC + slot_of_grp[grp] * GS + rank

    e_src = new_of_old[src]
    e_dst = new_of_old[dst]
    e_core = e_dst // NPC
    e_slot = (e_dst % NPC) // GS
    e_loc = e_dst % GS

    ekey = e_core * NSLOT + e_slot
    eord = np.argsort(ekey, kind="stable")
    skey = ekey[eord]
    cnts = np.bincount(ekey, minlength=C * NSLOT)
    starts = np.zeros(C * NSLOT, dtype=np.int64)
    starts[1:] = np.cumsum(cnts)[:-1]
    k = np.arange(M) - starts[skey]
    lane_p = k % P
    ch_glob = slot_start[skey % NSLOT] + k // P
    core_s = skey // NSLOT

    srcidx = np.zeros((C, P, NCH), dtype=np.int32)
    srcidx[core_s, lane_p, ch_glob] = e_src[eord].astype(np.int32)

    normhot = np.zeros((C, P, NCH, GS), dtype=np.float32)
    normhot[core_s, lane_p, ch_glob, e_loc[eord]] = norm[eord]

    xsrc = np.zeros((C, P, NCH, 8), dtype=np.float32)
    xsrc[core_s, lane_p, ch_glob, 0:5] = x[src[eord]]

    ghot = np.zeros((C, P, NSUB, G), dtype=np.float32)
    nn = new_of_old
    ghot[nn // NPC, nn % P, (nn % NPC) // P, batch.astype(np.int64)] = 1.0

    counts = np.bincount(batch.astype(np.int64), minlength=G).astype(np.float32)
    cinv = (1.0 / np.maximum(counts, 1.0)).reshape(1, G).astype(np.float32)

    meta = dict(NCH=NCH, chunks_per_slot=chunks_per_slot, slot_start=slot_start)
    arrays = dict(srcidx=srcidx, normhot=normhot, xsrc=xsrc, ghot=ghot, cinv=cinv)
    return meta, arrays


def _pack_weights(W1, b1, W2, b2, W3, b3, W4, b4, fcW, fcb):
    wp1 = np.zeros((33, 32), dtype=np.float32)
    wp1[0:5] = W1
    wp1[32] = b1
    wp2 = np.zeros((33, 64), dtype=np.float32)
    wp2[0:32] = W2
    wp2[32] = b2
    wp3 = np.zeros((65, 128), dtype=np.float32)
    wp3[0:64] = W3
    wp3[64] = b3
    return dict(
        wp1=wp1.astype(BF16),
        wp2=wp2.astype(BF16),
        wp3=wp3.astype(BF16),
        w4=W4.astype(BF16),
        b4b=b4.reshape(1, 256).astype(BF16),
        fcw=fcW.astype(np.float32),
        fcbt=fcb.reshape(NCLS, 1).astype(np.float32),
    )


def build_in_maps(inputs):
    meta, arr = _preprocess(
        np.asarray(inputs["x"], dtype=np.float32),
        np.asarray(inputs["edge_index"]),
        np.asarray(inputs["batch"]),
    )
    w = _pack_weights(
        *(np.asarray(inputs[k], dtype=np.float32) for k in (
            "W1", "b1", "W2", "b2", "W3", "b3", "W4", "b4", "fcW", "fcb"))
    )
    NCH = meta["NCH"]
    in_maps = []
    for c in range(C):
        m = dict(w)
        m["srcidx"] = arr["srcidx"][c]
        m["normhot"] = arr["normhot"][c].reshape(P, NCH * GS).astype(BF16)
        m["xsrc"] = arr["xsrc"][c].reshape(P, NCH * 8).astype(BF16)
        m["ghot"] = arr["ghot"][c].reshape(P, NSUB * G).astype(BF16)
        m["cinv"] = arr["cinv"]
        in_maps.append(m)
    return meta, in_maps


# -------------------------------------------------------------- device side


def build_program(meta, finalize=True):
    NCH = meta["NCH"]
    chunks_per_slot = meta["chunks_per_slot"]
    slot_start = meta["slot_start"]

    f32 = mybir.dt.float32
    bf16 = mybir.dt.bfloat16
    i32 = mybir.dt.int32
    AF = mybir.ActivationFunctionType
    OP = mybir.AluOpType
    groups = [list(range(C))]

    nc = Bacc("TRN2", target_bir_lowering=False, debug=False, num_devices=C)

    xsrc_d = nc.dram_tensor("xsrc", [P, NCH * 8], bf16, kind="ExternalInput")
    srcidx_d = nc.dram_tensor("srcidx", [P, NCH], i32, kind="ExternalInput")
    normhot_d = nc.dram_tensor("normhot", [P, NCH * GS], bf16, kind="ExternalInput")
    ghot_d = nc.dram_tensor("ghot", [P, NSUB * G], bf16, kind="ExternalInput")
    wp1_d = nc.dram_tensor("wp1", [33, 32], bf16, kind="ExternalInput")
    wp2_d = nc.dram_tensor("wp2", [33, 64], bf16, kind="ExternalInput")
    wp3_d = nc.dram_tensor("wp3", [65, 128], bf16, kind="ExternalInput")
    w4_d = nc.dram_tensor("w4", [128, 256], bf16, kind="ExternalInput")
    b4b_d = nc.dram_tensor("b4b", [1, 256], bf16, kind="ExternalInput")
    fcw_d = nc.dram_tensor("fcw", [256, NCLS], f32, kind="ExternalInput")
    fcbt_d = nc.dram_tensor("fcbt", [NCLS, 1], f32, kind="ExternalInput")
    cinv_d = nc.dram_tensor("cinv", [1, G], f32, kind="ExternalInput")
    out_d = nc.dram_tensor("out", [G, NCLS], f32, kind="ExternalOutput")

    with tile.TileContext(nc) as tc, ExitStack() as ctx:
        dram = ctx.enter_context(tc.tile_pool(name="dram", bufs=1, space="DRAM"))
        const = ctx.enter_context(tc.tile_pool(name="const", bufs=1))
        work = ctx.enter_context(tc.tile_pool(name="work", bufs=1))
        pp = ctx.enter_context(tc.tile_pool(name="ps", bufs=1, space="PSUM"))

        tin = [None] * 4
        tg = [None] * 4
        for li, d_out in enumerate(DIMS_OUT[:3]):
            tin[li] = dram.tile([NPC, d_out], bf16, tag=f"t{li}in", name=f"t{li}in")
            tg[li] = dram.tile([NT, d_out], bf16, tag=f"t{li}g", name=f"t{li}g",
                               addr_space="Shared")
        plin = dram.tile([256, G], f32, tag="plin", name="plin")
        plg = dram.tile([256, G], f32, tag="plg", name="plg", addr_space="Shared")

        # ---- constant loads
        def load(name, dten, shape, dtype):
            t = const.tile(shape, dtype, tag=name, name=name)
            nc.sync.dma_start(out=t[:], in_=dten[:])
            return t

        normhot_sb = load("normhot_sb", normhot_d, [P, NCH * GS], bf16)
        srcidx_sb = load("srcidx_sb", srcidx_d, [P, NCH], i32)
        xsrc_sb = load("xsrc_sb", xsrc_d, [P, NCH * 8], bf16)
        ghot_sb = load("ghot_sb", ghot_d, [P, NSUB * G], bf16)
        wp1_sb = load("wp1_sb", wp1_d, [33, 32], bf16)
        wp2_sb = load("wp2_sb", wp2_d, [33, 64], bf16)
        wp3_sb = load("wp3_sb", wp3_d, [65, 128], bf16)
        w4_sb = load("w4_sb", w4_d, [128, 256], bf16)
        b4b_sb = load("b4b_sb", b4b_d, [1, 256], bf16)
        fcb_sb = load("fcb_sb", fcbt_d, [NCLS, 1], f32)
        cinv_sb = load("cinv_sb", cinv_d, [1, G], f32)
        fcw_sb = const.tile([P, 2, NCLS], f32, tag="fcw_sb", name="fcw_sb")
        nc.sync.dma_start(out=fcw_sb[:, 0, :], in_=fcw_d[0:128, :])
        nc.sync.dma_start(out=fcw_sb[:, 1, :], in_=fcw_d[128:256, :])

        ones1_sb = const.tile([1, P], bf16, tag="ones1", name="ones1")
        nc.vector.memset(ones1_sb[:], 1.0)
        ones1f_sb = const.tile([1, P], f32, tag="ones1f", name="ones1f")
        nc.vector.memset(ones1f_sb[:], 1.0)
        ident_sb = const.tile([P, P], f32, tag="ident", name="ident")
        make_identity(nc, ident_sb[:])

        h4nm = const.tile([P, NSUB * 256], bf16, tag="h4nm", name="h4nm")

        # per-chunk slot id and first/last flags
        slot_of_chunk = np.repeat(np.arange(NSLOT), chunks_per_slot)
        is_first = np.zeros(NCH, dtype=bool)
        is_last = np.zeros(NCH, dtype=bool)
        is_first[slot_start] = True
        is_last[slot_start + chunks_per_slot - 1] = True

        def scatter_layer(li, d_in, lhsT_of_chunk, prefetch):
            """Accumulate aggT[d_in, NPC] for this layer; returns aggT tile."""
            k_rows = [33, 33, 65, 128][li]
            aggT = work.tile([k_rows, NPC], bf16, tag="aggT", bufs=2,
                             name=f"aggT{li}")
            if li == 0:
                # rows 5..31 must be exactly 0 (wp1 rows are 0 there, but
                # garbage*0 could be NaN); ones row sits at legal partition 32
                nc.vector.memset(aggT[:, :], 0.0)
                nc.vector.memset(aggT[32:33, :], 1.0)
            elif li < 3:
                nc.vector.memset(aggT[d_in:d_in + 1, :], 1.0)
            ps = None
            if prefetch is not None:
                prefetch(0)
            for ch in range(NCH):
                j = slot_of_chunk[ch]
                if is_first[ch] and j % 16 == 0:
                    ps = pp.tile([P, 512], f32, tag="ps_s", bufs=2,
                                 name=f"ps_s_{li}_{j}")
                if prefetch is not None and is_first[ch] and (j % 16 == 8):
                    # issue next gather batches spread through the block
                    pass
                col = (j % 16) * GS
                nc.tensor.matmul(
                    out=ps[:d_in, col:col + GS],
                    lhsT=lhsT_of_chunk(ch),
                    rhs=normhot_sb[:, ch * GS:(ch + 1) * GS],
                    start=is_first[ch],
                    stop=is_last[ch],
                )
                if is_last[ch] and (j % 16 == 15 or j == NSLOT - 1):
                    ncols = (j % 16 + 1) * GS
                    j0 = (j // 16) * 16
                    nc.scalar.copy(
                        out=aggT[:d_in, j0 * GS:j0 * GS + ncols],
                        in_=ps[:d_in, :ncols],
                    )
            return aggT

        def gathered_layer(li, d_in, table):
            """Indirect-gather h[src] in small batches; returns (lhsT_of_chunk, prefetch).

            Safety envelope (measured on HW): indirect SWDGE DMAs corrupt the
            gathered data once an instruction carries more than 256
            descriptors (descriptor staging wraps under drain backlog and
            clobbers in-flight descriptors), so batches are fixed at
            NB=2 chunks = 256 gathered rows per instruction.
            """
            NB = 2
            GBUFS = 6
            tiles = {}

            def prefetch(b):
                b0 = b * NB
                if b0 >= NCH or b in tiles:
                    return
                nb = min(NB, NCH - b0)
                # constant tile size across layers: one tag must always alias
                # same-size slots
                gt = work.tile([P, NB * 128], bf16, tag="gath", bufs=GBUFS,
                               name=f"gath_{li}_{b}")
                nc.gpsimd.indirect_dma_start(
                    out=gt[:, :nb * d_in],
                    out_offset=None,
                    in_=table[:],
                    in_offset=bass.IndirectOffsetOnAxis(
                        ap=srcidx_sb[:, b0:b0 + nb], axis=0),
                )
                tiles[b] = gt

            def lhsT_of_chunk(ch):
                b, o = divmod(ch, NB)
                for k in (1, 2, 3, 4):
                    prefetch(b + k)
                return tiles[b][:, o * d_in:(o + 1) * d_in]

            return lhsT_of_chunk, prefetch

        def transform_layer(li, aggT, k_rows, w_sb, d_out):
            """Per 128-node subtile: relu(aggT_sub.T @ W'); emit node-major."""
            for t in range(NSUB):
                pst = pp.tile([P, 256], f32, tag="ps_t", bufs=2,
                              name=f"ps_t_{li}_{t}")
                nc.tensor.matmul(
                    out=pst[:, :d_out],
                    lhsT=aggT[:k_rows, t * P:(t + 1) * P],
                    rhs=w_sb[:k_rows, :d_out],
                    start=True,
                    stop=(li < 3),
                )
                if li == 3:
                    nc.tensor.matmul(
                        out=pst[:, :d_out],
                        lhsT=ones1_sb[:1, :P],
                        rhs=b4b_sb[:1, :256],
                        start=False,
                        stop=True,
                    )
                    nc.scalar.activation(
                        out=h4nm[:, t * 256:(t + 1) * 256],
                        in_=pst[:, :256], func=AF.Relu)
                else:
                    hb = work.tile([P, 256], bf16, tag="hbuf", bufs=3,
                                   name=f"hbuf_{li}_{t}")
                    nc.scalar.activation(
                        out=hb[:, :d_out], in_=pst[:, :d_out], func=AF.Relu)
                    nc.sync.dma_start(
                        out=tin[li][t * P:(t + 1) * P, :], in_=hb[:, :d_out])

        # ---- layer 1 (sources preloaded on host)
        aggT = scatter_layer(0, 8, lambda ch: xsrc_sb[:, ch * 8:(ch + 1) * 8], None)
        transform_layer(0, aggT, 33, wp1_sb, 32)
        nc.gpsimd.collective_compute(
            "AllGather", mybir.AluOpType.bypass, replica_groups=groups,
            ins=[tin[0].opt()], outs=[tg[0].opt()])

        # ---- layers 2..4
        for li, (d_in, d_out, w_sb) in enumerate(
                [(32, 64, wp2_sb), (64, 128, wp3_sb), (128, 256, w4_sb)], start=1):
            lhsT_of_chunk, prefetch = gathered_layer(li, d_in, tg[li - 1])
            aggT = scatter_layer(li, d_in, lhsT_of_chunk, prefetch)
            k_rows = d_in + 1 if li < 3 else d_in
            transform_layer(li, aggT, k_rows, w_sb, d_out)
            if li < 3:
                nc.gpsimd.collective_compute(
                    "AllGather", mybir.AluOpType.bypass, replica_groups=groups,
                    ins=[tin[li].opt()], outs=[tg[li].opt()])

        # ---- pooling: pooled.T[256, 64] = sum_t h4nm_sub.T @ ghot_sub
        psAB = pp.tile([P, 2 * G], f32, tag="psAB", name="psAB")
        poolsb = work.tile([P, 2 * G], f32, tag="poolsb", name="poolsb")
        for h in range(2):  # one accumulation group per psum bank at a time
            pst = psAB[:, h * G:(h + 1) * G]
            for t in range(NSUB):
                nc.tensor.matmul(
                    out=pst[:, :G],
                    lhsT=h4nm[:, t * 256 + h * P:t * 256 + h * P + P],
                    rhs=ghot_sb[:, t * G:(t + 1) * G],
                    start=(t == 0),
                    stop=(t == NSUB - 1),
                )
            nc.scalar.copy(out=poolsb[:, h * G:(h + 1) * G], in_=pst[:, :G])
        nc.sync.dma_start(out=plin[0:128, :], in_=poolsb[:, 0:G])
        nc.sync.dma_start(out=plin[128:256, :], in_=poolsb[:, G:2 * G])
        nc.gpsimd.collective_compute(
            "AllReduce", mybir.AluOpType.add, replica_groups=groups,
            ins=[plin.opt()], outs=[plg.opt()])

        # ---- head (replicated on every core)
        pool2 = work.tile([P, 2, G], f32, tag="pool2", name="pool2")
        nc.sync.dma_start(out=pool2[:, 0, :], in_=plg[0:128, :])
        nc.sync.dma_start(out=pool2[:, 1, :], in_=plg[128:256, :])
        psHead = pp.tile([P, 512], f32, tag="psHead", name="psHead")
        psc = psHead[:, 128:128 + G]
        nc.tensor.matmul(out=psc[:, :G], lhsT=ones1f_sb[:1, :P],
                         rhs=cinv_sb[:1, :G], start=True, stop=True)
        for h in range(2):
            nc.vector.tensor_tensor(
                out=pool2[:, h, :], in0=pool2[:, h, :],
                in1=psc[:, :G], op=OP.mult)
        psh = psHead[:, 0:G]
        nc.tensor.matmul(out=psh[:NCLS, :G], lhsT=fcw_sb[:, 0, :],
                         rhs=pool2[:, 0, :], start=True, stop=False)
        nc.tensor.matmul(out=psh[:NCLS, :G], lhsT=fcw_sb[:, 1, :],
                         rhs=pool2[:, 1, :], start=False, stop=True)
        lt = work.tile([NCLS, G], f32, tag="lt", name="lt")
        nc.scalar.activation(out=lt[:NCLS, :G], in_=psh[:NCLS, :G],
                             func=AF.Identity, bias=fcb_sb[:NCLS, :1], scale=1.0)
        pstr = psHead[:, 192:192 + NCLS]
        nc.tensor.transpose(out=pstr[:G, :NCLS], in_=lt[:NCLS, :G],
                            identity=ident_sb[:NCLS, :NCLS])
        l2 = work.tile([G, NCLS], f32, tag="l2", name="l2")
        nc.scalar.copy(out=l2[:, :], in_=pstr[:G, :NCLS])

        mx = work.tile([G, 1], f32, tag="mx", name="mx")
        nc.vector.tensor_reduce(out=mx[:, :], in_=l2[:, :],
                                axis=mybir.AxisListType.X, op=OP.max)
        l2m = work.tile([G, NCLS], f32, tag="l2m", name="l2m")
        nc.vector.tensor_scalar_sub(out=l2m[:, :], in0=l2[:, :], scalar1=mx[:, :1])
        ex = work.tile([G, NCLS], f32, tag="ex", name="ex")
        nc.scalar.activation(out=ex[:, :], in_=l2m[:, :], func=AF.Exp)
        sm = work.tile([G, 1], f32, tag="sm", name="sm")
        nc.vector.tensor_reduce(out=sm[:, :], in_=ex[:, :],
                                axis=mybir.AxisListType.X, op=OP.add)
        lsm = work.tile([G, 1], f32, tag="lsm", name="lsm")
        nc.scalar.activation(out=lsm[:, :], in_=sm[:, :], func=AF.Ln)
        res = work.tile([G, NCLS], f32, tag="res", name="res")
        nc.vector.tensor_scalar_sub(out=res[:, :], in0=l2m[:, :], scalar1=lsm[:, :1])
        nc.sync.dma_start(out=out_d[:], in_=res[:, :])

    if finalize:
        nc.finalize()
    return nc


# ------------------------------------------------------------------- entry


def kernel(**inputs):
    from concourse.bass_utils import run_bass_kernel_spmd

    meta, in_maps = build_in_maps(inputs)
    nc = build_program(meta)
    r = run_bass_kernel_spmd(nc, in_maps, list(range(C)))
    return np.asarray(r.results[0]["out"], dtype=np.float32)


if __name__ == "__main__":
    rng = np.random.default_rng(0)
    demo = {
        "x": rng.standard_normal((N, 5), dtype=np.float32),
        "edge_index": rng.integers(0, N, (2, E)).astype(np.int64),
        "batch": np.sort(rng.integers(0, G, N)).astype(np.int64),
        "W1": rng.standard_normal((5, 32), dtype=np.float32) * 0.1,
        "b1": np.zeros(32, np.float32),
        "W2": rng.standard_normal((32, 64), dtype=np.float32) * 0.1,
        "b2": np.zeros(64, np.float32),
        "W3": rng.standard_normal((64, 128), dtype=np.float32) * 0.1,
        "b3": np.zeros(128, np.float32),
        "W4": rng.standard_normal((128, 256), dtype=np.float32) * 0.1,
        "b4": np.zeros(256, np.float32),
        "fcW": rng.standard_normal((256, 10), dtype=np.float32) * 0.1,
        "fcb": np.zeros(10, np.float32),
    }
    print(kernel(**demo))



# revision 15
# speedup vs baseline: 1.3221x; 1.0009x over previous
"""GCN (4-layer) + mean-pool + linear head on 8 Trainium2 cores.

Strategy: shard destination nodes across 8 cores (load-balanced by degree),
aggregate-before-transform (S(HW) == (SH)W), so edge gathers happen at the
layer *input* width (5/32/64/128 instead of 32/64/128/256).

Per layer, per core:
  gather   h[src] rows from the replicated node-feature table (indirect DMA)
  scatter  psum[d_in, 32dst] += gathered_chunk[128e, d_in].T @ onehot_norm[128e, 32]
  evac     psum -> aggT (feature-major SBUF [d_in(+1), 6272])
  transform per 128-node subtile: psum[128n, d_out] = aggT_sub[d_in+1,128].T @ W'
           (ones-row in aggT folds the bias), relu -> node-major bf16
  allgather the [6272, d_out] slice -> full table [50176, d_out] for next layer
Layer 4 keeps h4 in SBUF; pooling via matmul against per-graph one-hots,
AllReduce [256, 64], replicated fp32 head + log_softmax on every core.
"""

import sys

for _p in ("/opt/trn_rl_repo", "/opt/pypackages"):
    if _p not in sys.path:
        sys.path.insert(0, _p)

from contextlib import ExitStack

import numpy as np
import ml_dtypes

import concourse.bass as bass
import concourse.tile as tile
from concourse import mybir
from concourse.bacc import Bacc
from concourse.masks import make_identity

BF16 = ml_dtypes.bfloat16

N = 50000      # nodes
E = 800000     # edges (without self loops)
G = 64         # graphs
C = 8          # cores
P = 128
GS = 32        # dst-group size (psum scatter column block)
NPC = 6272     # padded nodes per core  (= 196*32 = 49*128)
NT = NPC * C   # 50176 padded total
NSLOT = NPC // GS   # 196 dst-groups per core
NGRP = NSLOT * C    # 1568 groups total
NSUB = NPC // P     # 49 128-node subtiles per core
NCLS = 10

DIMS_IN = [8, 32, 64, 128]    # scatter/gather width per layer (L1 padded 5->8)
DIMS_OUT = [32, 64, 128, 256]


# ---------------------------------------------------------------- host side


def _preprocess(x, edge_index, batch):
    src = np.concatenate([edge_index[0].astype(np.int64), np.arange(N, dtype=np.int64)])
    dst = np.concatenate([edge_index[1].astype(np.int64), np.arange(N, dtype=np.int64)])
    M = src.shape[0]

    deg = np.bincount(dst, minlength=N).astype(np.float32)
    dinv = 1.0 / np.sqrt(deg)
    norm = (dinv[src] * dinv[dst]).astype(np.float32)

    # deal nodes (sorted by degree desc) round-robin into NGRP groups of <=32
    order = np.argsort(-deg, kind="stable")
    grp = np.empty(N, dtype=np.int64)
    rank = np.empty(N, dtype=np.int64)
    pos = np.arange(N)
    grp[order] = pos % NGRP
    rank[order] = pos // NGRP

    gcnt = np.bincount(grp[dst], minlength=NGRP)  # incoming edges per group
    # deal groups (sorted by edge count desc) into 8 cores x 196 slots so the
    # 8 groups sharing a slot have near-equal counts (SPMD chunk uniformity)
    gorder = np.argsort(-gcnt, kind="stable")
    core_of_grp = np.empty(NGRP, dtype=np.int64)
    slot_of_grp = np.empty(NGRP, dtype=np.int64)
    gpos = np.arange(NGRP)
    core_of_grp[gorder] = gpos % C
    slot_of_grp[gorder] = gpos // C

    cnt_cs = np.zeros((C, NSLOT), dtype=np.int64)
    cnt_cs[core_of_grp, slot_of_grp] = gcnt
    chunks_per_slot = np.maximum(1, -(-cnt_cs.max(axis=0) // P)).astype(np.int64)
    slot_start = np.zeros(NSLOT, dtype=np.int64)
    slot_start[1:] = np.cumsum(chunks_per_slot)[:-1]
    NCH = int(chunks_per_slot.sum())

    new_of_old = core_of_grp[grp] * NPC + slot_of_grp[grp] * GS + rank

    e_src = new_of_old[src]
    e_dst = new_of_old[dst]
    e_core = e_dst // NPC
    e_slot = (e_dst % NPC) // GS
    e_loc = e_dst % GS

    ekey = e_core * NSLOT + e_slot
    eord = np.argsort(ekey, kind="stable")
    skey = ekey[eord]
    cnts = np.bincount(ekey, minlength=C * NSLOT)
    starts = np.zeros(C * NSLOT, dtype=np.int64)
    starts[1:] = np.cumsum(cnts)[:-1]
    k = np.arange(M) - starts[skey]
    lane_p = k % P
    ch_glob = slot_start[skey % NSLOT] + k // P
    core_s = skey // NSLOT

    srcidx = np.zeros((C, P, NCH), dtype=np.int32)
    srcidx[core_s, lane_p, ch_glob] = e_src[eord].astype(np.int32)

    normhot = np.zeros((C, P, NCH, GS), dtype=np.float32)
    normhot[core_s, lane_p, ch_glob, e_loc[eord]] = norm[eord]

    xsrc = np.zeros((C, P, NCH, 8), dtype=np.float32)
    xsrc[core_s, lane_p, ch_glob, 0:5] = x[src[eord]]

    ghot = np.zeros((C, P, NSUB, G), dtype=np.float32)
    nn = new_of_old
    ghot[nn // NPC, nn % P, (nn % NPC) // P, batch.astype(np.int64)] = 1.0

    counts = np.bincount(batch.astype(np.int64), minlength=G).astype(np.float32)
    cinv = (1.0 / np.maximum(counts, 1.0)).reshape(1, G).astype(np.float32)

    meta = dict(NCH=NCH, chunks_per_slot=chunks_per_slot, slot_start=slot_start)
    arrays = dict(srcidx=srcidx, normhot=normhot, xsrc=xsrc, ghot=ghot, cinv=cinv)
    return meta, arrays


def _pack_weights(W1, b1, W2, b2, W3, b3, W4, b4, fcW, fcb):
    wp1 = np.zeros((33, 32), dtype=np.float32)
    wp1[0:5] = W1
    wp1[32] = b1
    wp2 = np.zeros((33, 64), dtype=np.float32)
    wp2[0:32] = W2
    wp2[32] = b2
    wp3 = np.zeros((65, 128), dtype=np.float32)
    wp3[0:64] = W3
    wp3[64] = b3
    return dict(
        wp1=wp1.astype(BF16),
        wp2=wp2.astype(BF16),
        wp3=wp3.astype(BF16),
        w4=W4.astype(BF16),
        b4b=b4.reshape(1, 256).astype(BF16),
        fcw=fcW.astype(np.float32),
        fcbt=fcb.reshape(NCLS, 1).astype(np.float32),
    )


def build_in_maps(inputs):
    meta, arr = _preprocess(
        np.asarray(inputs["x"], dtype=np.float32),
        np.asarray(inputs["edge_index"]),
        np.asarray(inputs["batch"]),
    )
    w = _pack_weights(
        *(np.asarray(inputs[k], dtype=np.float32) for k in (
            "W1", "b1", "W2", "b2", "W3", "b3", "W4", "b4", "fcW", "fcb"))
    )
    NCH = meta["NCH"]
    in_maps = []
    for c in range(C):
        m = dict(w)
        m["srcidx"] = arr["srcidx"][c]
        m["normhot"] = arr["normhot"][c].reshape(P, NCH * GS).astype(BF16)
        m["xsrc"] = arr["xsrc"][c].reshape(P, NCH * 8).astype(BF16)
        m["ghot"] = arr["ghot"][c].reshape(P, NSUB * G).astype(BF16)
        m["cinv"] = arr["cinv"]
        in_maps.append(m)
    return meta, in_maps


# -------------------------------------------------------------- device side


def build_program(meta, finalize=True):
    NCH = meta["NCH"]
    chunks_per_slot = meta["chunks_per_slot"]
    slot_start = meta["slot_start"]

    f32 = mybir.dt.float32
    bf16 = mybir.dt.bfloat16
    i32 = mybir.dt.int32
    AF = mybir.ActivationFunctionType
    OP = mybir.AluOpType
    groups = [list(range(C))]

    nc = Bacc("TRN2", target_bir_lowering=False, debug=False, num_devices=C)

    xsrc_d = nc.dram_tensor("xsrc", [P, NCH * 8], bf16, kind="ExternalInput")
    srcidx_d = nc.dram_tensor("srcidx", [P, NCH], i32, kind="ExternalInput")
    normhot_d = nc.dram_tensor("normhot", [P, NCH * GS], bf16, kind="ExternalInput")
    ghot_d = nc.dram_tensor("ghot", [P, NSUB * G], bf16, kind="ExternalInput")
    wp1_d = nc.dram_tensor("wp1", [33, 32], bf16, kind="ExternalInput")
    wp2_d = nc.dram_tensor("wp2", [33, 64], bf16, kind="ExternalInput")
    wp3_d = nc.dram_tensor("wp3", [65, 128], bf16, kind="ExternalInput")
    w4_d = nc.dram_tensor("w4", [128, 256], bf16, kind="ExternalInput")
    b4b_d = nc.dram_tensor("b4b", [1, 256], bf16, kind="ExternalInput")
    fcw_d = nc.dram_tensor("fcw", [256, NCLS], f32, kind="ExternalInput")
    fcbt_d = nc.dram_tensor("fcbt", [NCLS, 1], f32, kind="ExternalInput")
    cinv_d = nc.dram_tensor("cinv", [1, G], f32, kind="ExternalInput")
    out_d = nc.dram_tensor("out", [G, NCLS], f32, kind="ExternalOutput")

    with tile.TileContext(nc) as tc, ExitStack() as ctx:
        dram = ctx.enter_context(tc.tile_pool(name="dram", bufs=1, space="DRAM"))
        const = ctx.enter_context(tc.tile_pool(name="const", bufs=1))
        work = ctx.enter_context(tc.tile_pool(name="work", bufs=1))
        pp = ctx.enter_context(tc.tile_pool(name="ps", bufs=1, space="PSUM"))

        tin = [None] * 4
        tg = [None] * 4
        for li, d_out in enumerate(DIMS_OUT[:3]):
            tin[li] = dram.tile([NPC, d_out], bf16, tag=f"t{li}in", name=f"t{li}in")
            tg[li] = dram.tile([NT, d_out], bf16, tag=f"t{li}g", name=f"t{li}g",
                               addr_space="Shared")
        plin = dram.tile([256, G], f32, tag="plin", name="plin")
        plg = dram.tile([256, G], f32, tag="plg", name="plg", addr_space="Shared")

        # ---- constant loads
        def load(name, dten, shape, dtype):
            t = const.tile(shape, dtype, tag=name, name=name)
            nc.sync.dma_start(out=t[:], in_=dten[:])
            return t

        normhot_sb = load("normhot_sb", normhot_d, [P, NCH * GS], bf16)
        srcidx_sb = load("srcidx_sb", srcidx_d, [P, NCH], i32)
        xsrc_sb = load("xsrc_sb", xsrc_d, [P, NCH * 8], bf16)
        ghot_sb = load("ghot_sb", ghot_d, [P, NSUB * G], bf16)
        wp1_sb = load("wp1_sb", wp1_d, [33, 32], bf16)
        wp2_sb = load("wp2_sb", wp2_d, [33, 64], bf16)
        wp3_sb = load("wp3_sb", wp3_d, [65, 128], bf16)
        w4_sb = load("w4_sb", w4_d, [128, 256], bf16)
        b4b_sb = load("b4b_sb", b4b_d, [1, 256], bf16)
        fcb_sb = load("fcb_sb", fcbt_d, [NCLS, 1], f32)
        cinv_sb = load("cinv_sb", cinv_d, [1, G], f32)
        fcw_sb = const.tile([P, 2, NCLS], f32, tag="fcw_sb", name="fcw_sb")
        nc.sync.dma_start(out=fcw_sb[:, 0, :], in_=fcw_d[0:128, :])
        nc.sync.dma_start(out=fcw_sb[:, 1, :], in_=fcw_d[128:256, :])

        ones1_sb = const.tile([1, P], bf16, tag="ones1", name="ones1")
        nc.vector.memset(ones1_sb[:], 1.0)
        ones1f_sb = const.tile([1, P], f32, tag="ones1f", name="ones1f")
        nc.vector.memset(ones1f_sb[:], 1.0)
        ident_sb = const.tile([P, P], f32, tag="ident", name="ident")
        make_identity(nc, ident_sb[:])

        h4nm = const.tile([P, NSUB * 256], bf16, tag="h4nm", name="h4nm")

        # per-chunk slot id and first/last flags
        slot_of_chunk = np.repeat(np.arange(NSLOT), chunks_per_slot)
        is_first = np.zeros(NCH, dtype=bool)
        is_last = np.zeros(NCH, dtype=bool)
        is_first[slot_start] = True
        is_last[slot_start + chunks_per_slot - 1] = True

        def scatter_layer(li, d_in, lhsT_of_chunk, prefetch):
            """Accumulate aggT[d_in, NPC] for this layer; returns aggT tile."""
            k_rows = [33, 33, 65, 128][li]
            aggT = work.tile([k_rows, NPC], bf16, tag="aggT", bufs=2,
                             name=f"aggT{li}")
            if li == 0:
                # rows 5..31 must be exactly 0 (wp1 rows are 0 there, but
                # garbage*0 could be NaN); ones row sits at legal partition 32
                nc.vector.memset(aggT[:, :], 0.0)
                nc.vector.memset(aggT[32:33, :], 1.0)
            elif li < 3:
                nc.vector.memset(aggT[d_in:d_in + 1, :], 1.0)
            ps = None
            if prefetch is not None:
                prefetch(0)
            for ch in range(NCH):
                j = slot_of_chunk[ch]
                if is_first[ch] and j % 16 == 0:
                    ps = pp.tile([P, 512], f32, tag="ps_s", bufs=2,
                                 name=f"ps_s_{li}_{j}")
                if prefetch is not None and is_first[ch] and (j % 16 == 8):
                    # issue next gather batches spread through the block
                    pass
                col = (j % 16) * GS
                nc.tensor.matmul(
                    out=ps[:d_in, col:col + GS],
                    lhsT=lhsT_of_chunk(ch),
                    rhs=normhot_sb[:, ch * GS:(ch + 1) * GS],
                    start=is_first[ch],
                    stop=is_last[ch],
                )
                if is_last[ch] and (j % 16 == 15 or j == NSLOT - 1):
                    ncols = (j % 16 + 1) * GS
                    j0 = (j // 16) * 16
                    nc.scalar.copy(
                        out=aggT[:d_in, j0 * GS:j0 * GS + ncols],
                        in_=ps[:d_in, :ncols],
                    )
            return aggT

        def gathered_layer(li, d_in, table):
            """Indirect-gather h[src] in small batches; returns (lhsT_of_chunk, prefetch).

            Safety envelope (measured on HW): indirect SWDGE DMAs corrupt the
            gathered data once an instruction carries more than 256
            descriptors (descriptor staging wraps under drain backlog and
            clobbers in-flight descriptors), so batches are fixed at
            NB=2 chunks = 256 gathered rows per instruction.
            """
            NB = 2
            GBUFS = 6
            tiles = {}

            def prefetch(b):
                b0 = b * NB
                if b0 >= NCH or b in tiles:
                    return
                nb = min(NB, NCH - b0)
                # constant tile size across layers: one tag must always alias
                # same-size slots
                gt = work.tile([P, NB * 128], bf16, tag="gath", bufs=GBUFS,
                               name=f"gath_{li}_{b}")
                nc.gpsimd.indirect_dma_start(
                    out=gt[:, :nb * d_in],
                    out_offset=None,
                    in_=table[:],
                    in_offset=bass.IndirectOffsetOnAxis(
                        ap=srcidx_sb[:, b0:b0 + nb], axis=0),
                )
                tiles[b] = gt

            def lhsT_of_chunk(ch):
                b, o = divmod(ch, NB)
                for k in (1, 2, 3, 4):
                    prefetch(b + k)
                return tiles[b][:, o * d_in:(o + 1) * d_in]

            return lhsT_of_chunk, prefetch

        def transform_layer(li, aggT, k_rows, w_sb, d_out):
            """Per 128-node subtile: relu(aggT_sub.T @ W'); emit node-major."""
            for t in range(NSUB):
                pst = pp.tile([P, 256], f32, tag="ps_t", bufs=2,
                              name=f"ps_t_{li}_{t}")
                nc.tensor.matmul(
                    out=pst[:, :d_out],
                    lhsT=aggT[:k_rows, t * P:(t + 1) * P],
                    rhs=w_sb[:k_rows, :d_out],
                    start=True,
                    stop=(li < 3),
                )
                if li == 3:
                    nc.tensor.matmul(
                        out=pst[:, :d_out],
                        lhsT=ones1_sb[:1, :P],
                        rhs=b4b_sb[:1, :256],
                        start=False,
                        stop=True,
                    )
                    nc.scalar.activation(
                        out=h4nm[:, t * 256:(t + 1) * 256],
                        in_=pst[:, :256], func=AF.Relu)
                else:
                    hb = work.tile([P, 256], bf16, tag="hbuf", bufs=3,
                                   name=f"hbuf_{li}_{t}")
                    nc.scalar.activation(
                        out=hb[:, :d_out], in_=pst[:, :d_out], func=AF.Relu)
                    nc.sync.dma_start(
                        out=tin[li][t * P:(t + 1) * P, :], in_=hb[:, :d_out])

        # ---- layer 1 (sources preloaded on host)
        aggT = scatter_layer(0, 8, lambda ch: xsrc_sb[:, ch * 8:(ch + 1) * 8], None)
        transform_layer(0, aggT, 33, wp1_sb, 32)
        nc.gpsimd.collective_compute(
            "AllGather", mybir.AluOpType.bypass, replica_groups=groups,
            ins=[tin[0].opt()], outs=[tg[0].opt()])

        # ---- layers 2..4
        for li, (d_in, d_out, w_sb) in enumerate(
                [(32, 64, wp2_sb), (64, 128, wp3_sb), (128, 256, w4_sb)], start=1):
            lhsT_of_chunk, prefetch = gathered_layer(li, d_in, tg[li - 1])
            aggT = scatter_layer(li, d_in, lhsT_of_chunk, prefetch)
            k_rows = d_in + 1 if li < 3 else d_in
            transform_layer(li, aggT, k_rows, w_sb, d_out)
            if li < 3:
                nc.gpsimd.collective_compute(
                    "AllGather", mybir.AluOpType.bypass, replica_groups=groups,
                    ins=[tin[li].opt()], outs=[tg[li].opt()])

        # ---- pooling: pooled.T[256, 64] = sum_t h4nm_sub.T @ ghot_sub
        psAB = pp.tile([P, 2 * G], f32, tag="psAB", name="psAB")
        poolsb = work.tile([P, 2 * G], f32, tag="poolsb", name="poolsb")
        for h in range(2):  # one accumulation group per psum bank at a time
            pst = psAB[:, h * G:(h + 1) * G]
            for t in range(NSUB):
                nc.tensor.matmul(
                    out=pst[:, :G],
                    lhsT=h4nm[:, t * 256 + h * P:t * 256 + h * P + P],
                    rhs=ghot_sb[:, t * G:(t + 1) * G],
                    start=(t == 0),
                    stop=(t == NSUB - 1),
                )
            nc.scalar.copy(out=poolsb[:, h * G:(h + 1) * G], in_=pst[:, :G])
        nc.sync.dma_start(out=plin[0:128, :], in_=poolsb[:, 0:G])
        nc.sync.dma_start(out=plin[128:256, :], in_=poolsb[:, G:2 * G])
        nc.gpsimd.collective_compute(
            "AllReduce", mybir.AluOpType.add, replica_groups=groups,
            ins=[plin.opt()], outs=[plg.opt()])

        # ---- head (replicated on every core)
        pool2 = work.tile([P, 2, G], f32, tag="pool2", name="pool2")
        nc.sync.dma_start(out=pool2[:, 0, :], in_=plg[0:128, :])
        nc.sync.dma_start(out=pool2[:, 1, :], in_=plg[128:256, :])
        psHead = pp.tile([P, 512], f32, tag="psHead", name="psHead")
        psc = psHead[:, 128:128 + G]
        nc.tensor.matmul(out=psc[:, :G], lhsT=ones1f_sb[:1, :P],
                         rhs=cinv_sb[:1, :G], start=True, stop=True)
        for h in range(2):
            nc.vector.tensor_tensor(
                out=pool2[:, h, :], in0=pool2[:, h, :],
                in1=psc[:, :G], op=OP.mult)
        psh = psHead[:, 0:G]
        nc.tensor.matmul(out=psh[:NCLS, :G], lhsT=fcw_sb[:, 0, :],
                         rhs=pool2[:, 0, :], start=True, stop=False)
        nc.tensor.matmul(out=psh[:NCLS, :G], lhsT=fcw_sb[:, 1, :],
                         rhs=pool2[:, 1, :], start=False, stop=True)
        lt = work.tile([NCLS, G], f32, tag="lt", name="lt")
        nc.scalar.activation(out=lt[:NCLS, :G], in_=psh[:NCLS, :G],
                             func=AF.Identity, bias=fcb_sb[:NCLS, :1], scale=1.0)
        pstr = psHead[:, 192:192 + NCLS]
        nc.tensor.transpose(out=pstr[:G, :NCLS], in_=lt[:NCLS, :G],
                            identity=ident_sb[:NCLS, :NCLS])
        l2 = work.tile([G, NCLS], f32, tag="l2", name="l2")
        nc.scalar.copy(out=l2[:, :], in_=pstr[:G, :NCLS])

        mx = work.tile([G, 1], f32, tag="mx", name="mx")
        nc.vector.tensor_reduce(out=mx[:, :], in_=l2[:, :],
                                axis=mybir.AxisListType.X, op=OP.max)
        l2m = work.tile([G, NCLS], f32, tag="l2m", name="l2m")
        nc.vector.tensor_scalar_sub(out=l2m[:, :], in0=l2[:, :], scalar1=mx[:, :1])
        ex = work.tile([G, NCLS], f32, tag="ex", name="ex")
        nc.scalar.activation(out=ex[:, :], in_=l2m[:, :], func=AF.Exp)
        sm = work.tile([G, 1], f32, tag="sm", name="sm")
        nc.vector.tensor_reduce(out=sm[:, :], in_=ex[:, :],
                                axis=mybir.AxisListType.X, op=OP.add)
        lsm = work.tile([G, 1], f32, tag="lsm", name="lsm")
        nc.scalar.activation(out=lsm[:, :], in_=sm[:, :], func=AF.Ln)
        res = work.tile([G, NCLS], f32, tag="res", name="res")
        nc.vector.tensor_scalar_sub(out=res[:, :], in0=l2m[:, :], scalar1=lsm[:, :1])
        nc.sync.dma_start(out=out_d[:], in_=res[:, :])

    if finalize:
        nc.finalize()
    return nc


# ------------------------------------------------------------------- entry


def kernel(**inputs):
    from concourse.bass_utils import run_bass_kernel_spmd

    meta, in_maps = build_in_maps(inputs)
    nc = build_program(meta)
    r = run_bass_kernel_spmd(nc, in_maps, list(range(C)))
    return np.asarray(r.results[0]["out"], dtype=np.float32)


if __name__ == "__main__":
    rng = np.random.default_rng(0)
    demo = {
        "x": rng.standard_normal((N, 5), dtype=np.float32),
        "edge_index": rng.integers(0, N, (2, E)).astype(np.int64),
        "batch": np.sort(rng.integers(0, G, N)).astype(np.int64),
        "W1": rng.standard_normal((5, 32), dtype=np.float32) * 0.1,
        "b1": np.zeros(32, np.float32),
        "W2": rng.standard_normal((32, 64), dtype=np.float32) * 0.1,
        "b2": np.zeros(64, np.float32),
        "W3": rng.standard_normal((64, 128), dtype=np.float32) * 0.1,
        "b3": np.zeros(128, np.float32),
        "W4": rng.standard_normal((128, 256), dtype=np.float32) * 0.1,
        "b4": np.zeros(256, np.float32),
        "fcW": rng.standard_normal((256, 10), dtype=np.float32) * 0.1,
        "fcb": np.zeros(10, np.float32),
    }
    print(kernel(**demo))



# revision 17
# speedup vs baseline: 1.3223x; 1.0002x over previous
"""GCN (4-layer) + mean-pool + linear head on 8 Trainium2 cores.

Strategy: shard destination nodes across 8 cores (load-balanced by degree),
aggregate-before-transform (S(HW) == (SH)W), so edge gathers happen at the
layer *input* width (5/32/64/128 instead of 32/64/128/256).

Per layer, per core:
  gather   h[src] rows from the replicated node-feature table (indirect DMA)
  scatter  psum[d_in, 32dst] += gathered_chunk[128e, d_in].T @ onehot_norm[128e, 32]
  evac     psum -> aggT (feature-major SBUF [d_in(+1), 6272])
  transform per 128-node subtile: psum[128n, d_out] = aggT_sub[d_in+1,128].T @ W'
           (ones-row in aggT folds the bias), relu -> node-major bf16
  allgather the [6272, d_out] slice -> full table [50176, d_out] for next layer
Layer 4 keeps h4 in SBUF; pooling via matmul against per-graph one-hots,
AllReduce [256, 64], replicated fp32 head + log_softmax on every core.
"""

import sys

for _p in ("/opt/trn_rl_repo", "/opt/pypackages"):
    if _p not in sys.path:
        sys.path.insert(0, _p)

from contextlib import ExitStack

import numpy as np
import ml_dtypes

import concourse.bass as bass
import concourse.tile as tile
from concourse import mybir
from concourse.bacc import Bacc
from concourse.masks import make_identity

BF16 = ml_dtypes.bfloat16

N = 50000      # nodes
E = 800000     # edges (without self loops)
G = 64         # graphs
C = 8          # cores
P = 128
GS = 32        # dst-group size (psum scatter column block)
NPC = 6272     # padded nodes per core  (= 196*32 = 49*128)
NT = NPC * C   # 50176 padded total
NSLOT = NPC // GS   # 196 dst-groups per core
NGRP = NSLOT * C    # 1568 groups total
NSUB = NPC // P     # 49 128-node subtiles per core
NCLS = 10

DIMS_IN = [8, 32, 64, 128]    # scatter/gather width per layer (L1 padded 5->8)
DIMS_OUT = [32, 64, 128, 256]


# ---------------------------------------------------------------- host side


def _preprocess(x, edge_index, batch):
    src = np.concatenate([edge_index[0].astype(np.int64), np.arange(N, dtype=np.int64)])
    dst = np.concatenate([edge_index[1].astype(np.int64), np.arange(N, dtype=np.int64)])
    M = src.shape[0]

    deg = np.bincount(dst, minlength=N).astype(np.float32)
    dinv = 1.0 / np.sqrt(deg)
    norm = (dinv[src] * dinv[dst]).astype(np.float32)

    # deal nodes (sorted by degree desc) round-robin into NGRP groups of <=32
    order = np.argsort(-deg, kind="stable")
    grp = np.empty(N, dtype=np.int64)
    rank = np.empty(N, dtype=np.int64)
    pos = np.arange(N)
    grp[order] = pos % NGRP
    rank[order] = pos // NGRP

    gcnt = np.bincount(grp[dst], minlength=NGRP)  # incoming edges per group
    # deal groups (sorted by edge count desc) into 8 cores x 196 slots so the
    # 8 groups sharing a slot have near-equal counts (SPMD chunk uniformity)
    gorder = np.argsort(-gcnt, kind="stable")
    core_of_grp = np.empty(NGRP, dtype=np.int64)
    slot_of_grp = np.empty(NGRP, dtype=np.int64)
    gpos = np.arange(NGRP)
    core_of_grp[gorder] = gpos % C
    slot_of_grp[gorder] = gpos // C

    cnt_cs = np.zeros((C, NSLOT), dtype=np.int64)
    cnt_cs[core_of_grp, slot_of_grp] = gcnt
    chunks_per_slot = np.maximum(1, -(-cnt_cs.max(axis=0) // P)).astype(np.int64)
    slot_start = np.zeros(NSLOT, dtype=np.int64)
    slot_start[1:] = np.cumsum(chunks_per_slot)[:-1]
    NCH = int(chunks_per_slot.sum())

    new_of_old = core_of_grp[grp] * NPC + slot_of_grp[grp] * GS + rank

    e_src = new_of_old[src]
    e_dst = new_of_old[dst]
    e_core = e_dst // NPC
    e_slot = (e_dst % NPC) // GS
    e_loc = e_dst % GS

    ekey = e_core * NSLOT + e_slot
    eord = np.argsort(ekey, kind="stable")
    skey = ekey[eord]
    cnts = np.bincount(ekey, minlength=C * NSLOT)
    starts = np.zeros(C * NSLOT, dtype=np.int64)
    starts[1:] = np.cumsum(cnts)[:-1]
    k = np.arange(M) - starts[skey]
    lane_p = k % P
    ch_glob = slot_start[skey % NSLOT] + k // P
    core_s = skey // NSLOT

    srcidx = np.zeros((C, P, NCH), dtype=np.int32)
    srcidx[core_s, lane_p, ch_glob] = e_src[eord].astype(np.int32)

    normhot = np.zeros((C, P, NCH, GS), dtype=np.float32)
    normhot[core_s, lane_p, ch_glob, e_loc[eord]] = norm[eord]

    xsrc = np.zeros((C, P, NCH, 8), dtype=np.float32)
    xsrc[core_s, lane_p, ch_glob, 0:5] = x[src[eord]]

    ghot = np.zeros((C, P, NSUB, G), dtype=np.float32)
    nn = new_of_old
    ghot[nn // NPC, nn % P, (nn % NPC) // P, batch.astype(np.int64)] = 1.0

    counts = np.bincount(batch.astype(np.int64), minlength=G).astype(np.float32)
    cinv = (1.0 / np.maximum(counts, 1.0)).reshape(1, G).astype(np.float32)

    meta = dict(NCH=NCH, chunks_per_slot=chunks_per_slot, slot_start=slot_start)
    arrays = dict(srcidx=srcidx, normhot=normhot, xsrc=xsrc, ghot=ghot, cinv=cinv)
    return meta, arrays


def _pack_weights(W1, b1, W2, b2, W3, b3, W4, b4, fcW, fcb):
    wp1 = np.zeros((33, 32), dtype=np.float32)
    wp1[0:5] = W1
    wp1[32] = b1
    wp2 = np.zeros((33, 64), dtype=np.float32)
    wp2[0:32] = W2
    wp2[32] = b2
    wp3 = np.zeros((65, 128), dtype=np.float32)
    wp3[0:64] = W3
    wp3[64] = b3
    return dict(
        wp1=wp1.astype(BF16),
        wp2=wp2.astype(BF16),
        wp3=wp3.astype(BF16),
        w4=W4.astype(BF16),
        b4b=b4.reshape(1, 256).astype(BF16),
        fcw=fcW.astype(np.float32),
        fcbt=fcb.reshape(NCLS, 1).astype(np.float32),
    )


def build_in_maps(inputs):
    meta, arr = _preprocess(
        np.asarray(inputs["x"], dtype=np.float32),
        np.asarray(inputs["edge_index"]),
        np.asarray(inputs["batch"]),
    )
    w = _pack_weights(
        *(np.asarray(inputs[k], dtype=np.float32) for k in (
            "W1", "b1", "W2", "b2", "W3", "b3", "W4", "b4", "fcW", "fcb"))
    )
    NCH = meta["NCH"]
    in_maps = []
    for c in range(C):
        m = dict(w)
        m["srcidx"] = arr["srcidx"][c]
        m["normhot"] = arr["normhot"][c].reshape(P, NCH * GS).astype(BF16)
        m["xsrc"] = arr["xsrc"][c].reshape(P, NCH * 8).astype(BF16)
        m["ghot"] = arr["ghot"][c].reshape(P, NSUB * G).astype(BF16)
        m["cinv"] = arr["cinv"]
        in_maps.append(m)
    return meta, in_maps


# -------------------------------------------------------------- device side


def build_program(meta, finalize=True):
    NCH = meta["NCH"]
    chunks_per_slot = meta["chunks_per_slot"]
    slot_start = meta["slot_start"]

    f32 = mybir.dt.float32
    bf16 = mybir.dt.bfloat16
    i32 = mybir.dt.int32
    AF = mybir.ActivationFunctionType
    OP = mybir.AluOpType
    groups = [list(range(C))]

    nc = Bacc("TRN2", target_bir_lowering=False, debug=False, num_devices=C)

    xsrc_d = nc.dram_tensor("xsrc", [P, NCH * 8], bf16, kind="ExternalInput")
    srcidx_d = nc.dram_tensor("srcidx", [P, NCH], i32, kind="ExternalInput")
    normhot_d = nc.dram_tensor("normhot", [P, NCH * GS], bf16, kind="ExternalInput")
    ghot_d = nc.dram_tensor("ghot", [P, NSUB * G], bf16, kind="ExternalInput")
    wp1_d = nc.dram_tensor("wp1", [33, 32], bf16, kind="ExternalInput")
    wp2_d = nc.dram_tensor("wp2", [33, 64], bf16, kind="ExternalInput")
    wp3_d = nc.dram_tensor("wp3", [65, 128], bf16, kind="ExternalInput")
    w4_d = nc.dram_tensor("w4", [128, 256], bf16, kind="ExternalInput")
    b4b_d = nc.dram_tensor("b4b", [1, 256], bf16, kind="ExternalInput")
    fcw_d = nc.dram_tensor("fcw", [256, NCLS], f32, kind="ExternalInput")
    fcbt_d = nc.dram_tensor("fcbt", [NCLS, 1], f32, kind="ExternalInput")
    cinv_d = nc.dram_tensor("cinv", [1, G], f32, kind="ExternalInput")
    out_d = nc.dram_tensor("out", [G, NCLS], f32, kind="ExternalOutput")

    with tile.TileContext(nc) as tc, ExitStack() as ctx:
        dram = ctx.enter_context(tc.tile_pool(name="dram", bufs=1, space="DRAM"))
        const = ctx.enter_context(tc.tile_pool(name="const", bufs=1))
        work = ctx.enter_context(tc.tile_pool(name="work", bufs=1))
        pp = ctx.enter_context(tc.tile_pool(name="ps", bufs=1, space="PSUM"))

        tin = [None] * 4
        tg = [None] * 4
        for li, d_out in enumerate(DIMS_OUT[:3]):
            tin[li] = dram.tile([NPC, d_out], bf16, tag=f"t{li}in", name=f"t{li}in")
            tg[li] = dram.tile([NT, d_out], bf16, tag=f"t{li}g", name=f"t{li}g",
                               addr_space="Shared")
        plin = dram.tile([256, G], f32, tag="plin", name="plin")
        plg = dram.tile([256, G], f32, tag="plg", name="plg", addr_space="Shared")

        # ---- constant loads
        def load(name, dten, shape, dtype):
            t = const.tile(shape, dtype, tag=name, name=name)
            nc.sync.dma_start(out=t[:], in_=dten[:])
            return t

        normhot_sb = load("normhot_sb", normhot_d, [P, NCH * GS], bf16)
        srcidx_sb = load("srcidx_sb", srcidx_d, [P, NCH], i32)
        xsrc_sb = load("xsrc_sb", xsrc_d, [P, NCH * 8], bf16)
        ghot_sb = load("ghot_sb", ghot_d, [P, NSUB * G], bf16)
        wp1_sb = load("wp1_sb", wp1_d, [33, 32], bf16)
        wp2_sb = load("wp2_sb", wp2_d, [33, 64], bf16)
        wp3_sb = load("wp3_sb", wp3_d, [65, 128], bf16)
        w4_sb = load("w4_sb", w4_d, [128, 256], bf16)
        b4b_sb = load("b4b_sb", b4b_d, [1, 256], bf16)
        fcb_sb = load("fcb_sb", fcbt_d, [NCLS, 1], f32)
        cinv_sb = load("cinv_sb", cinv_d, [1, G], f32)
        fcw_sb = const.tile([P, 2, NCLS], f32, tag="fcw_sb", name="fcw_sb")
        nc.sync.dma_start(out=fcw_sb[:, 0, :], in_=fcw_d[0:128, :])
        nc.sync.dma_start(out=fcw_sb[:, 1, :], in_=fcw_d[128:256, :])

        ones1_sb = const.tile([1, P], bf16, tag="ones1", name="ones1")
        nc.vector.memset(ones1_sb[:], 1.0)
        ones1f_sb = const.tile([1, P], f32, tag="ones1f", name="ones1f")
        nc.vector.memset(ones1f_sb[:], 1.0)
        ident_sb = const.tile([P, P], f32, tag="ident", name="ident")
        make_identity(nc, ident_sb[:])

        h4nm = const.tile([P, NSUB * 256], bf16, tag="h4nm", name="h4nm")

        # per-chunk slot id and first/last flags
        slot_of_chunk = np.repeat(np.arange(NSLOT), chunks_per_slot)
        is_first = np.zeros(NCH, dtype=bool)
        is_last = np.zeros(NCH, dtype=bool)
        is_first[slot_start] = True
        is_last[slot_start + chunks_per_slot - 1] = True

        def scatter_layer(li, d_in, lhsT_of_chunk, prefetch):
            """Accumulate aggT[d_in, NPC] for this layer; returns aggT tile."""
            k_rows = [33, 33, 65, 128][li]
            aggT = work.tile([k_rows, NPC], bf16, tag="aggT", bufs=2,
                             name=f"aggT{li}")
            if li == 0:
                # rows 5..31 must be exactly 0 (wp1 rows are 0 there, but
                # garbage*0 could be NaN); ones row sits at legal partition 32
                nc.vector.memset(aggT[:, :], 0.0)
                nc.vector.memset(aggT[32:33, :], 1.0)
            elif li < 3:
                nc.vector.memset(aggT[d_in:d_in + 1, :], 1.0)
            ps = None
            if prefetch is not None:
                prefetch(0)
            for ch in range(NCH):
                j = slot_of_chunk[ch]
                if is_first[ch] and j % 16 == 0:
                    ps = pp.tile([P, 512], f32, tag="ps_s", bufs=2,
                                 name=f"ps_s_{li}_{j}")
                if prefetch is not None and is_first[ch] and (j % 16 == 8):
                    # issue next gather batches spread through the block
                    pass
                col = (j % 16) * GS
                nc.tensor.matmul(
                    out=ps[:d_in, col:col + GS],
                    lhsT=lhsT_of_chunk(ch),
                    rhs=normhot_sb[:, ch * GS:(ch + 1) * GS],
                    start=is_first[ch],
                    stop=is_last[ch],
                )
                if is_last[ch] and (j % 16 == 15 or j == NSLOT - 1):
                    ncols = (j % 16 + 1) * GS
                    j0 = (j // 16) * 16
                    nc.scalar.copy(
                        out=aggT[:d_in, j0 * GS:j0 * GS + ncols],
                        in_=ps[:d_in, :ncols],
                    )
            return aggT

        def gathered_layer(li, d_in, table):
            """Indirect-gather h[src] in small batches; returns (lhsT_of_chunk, prefetch).

            Safety envelope (measured on HW): indirect SWDGE DMAs corrupt the
            gathered data once an instruction carries more than 256
            descriptors (descriptor staging wraps under drain backlog and
            clobbers in-flight descriptors), so batches are fixed at
            NB=2 chunks = 256 gathered rows per instruction.
            """
            NB = 2
            GBUFS = 6
            tiles = {}

            def prefetch(b):
                b0 = b * NB
                if b0 >= NCH or b in tiles:
                    return
                nb = min(NB, NCH - b0)
                # constant tile size across layers: one tag must always alias
                # same-size slots
                gt = work.tile([P, NB * 128], bf16, tag="gath", bufs=GBUFS,
                               name=f"gath_{li}_{b}")
                nc.gpsimd.indirect_dma_start(
                    out=gt[:, :nb * d_in],
                    out_offset=None,
                    in_=table[:],
                    in_offset=bass.IndirectOffsetOnAxis(
                        ap=srcidx_sb[:, b0:b0 + nb], axis=0),
                )
                tiles[b] = gt

            def lhsT_of_chunk(ch):
                b, o = divmod(ch, NB)
                for k in (1, 2, 3, 4):
                    prefetch(b + k)
                return tiles[b][:, o * d_in:(o + 1) * d_in]

            return lhsT_of_chunk, prefetch

        def transform_layer(li, aggT, k_rows, w_sb, d_out):
            """Per 128-node subtile: relu(aggT_sub.T @ W'); emit node-major."""
            for t in range(NSUB):
                pst = pp.tile([P, 256], f32, tag="ps_t", bufs=2,
                              name=f"ps_t_{li}_{t}")
                nc.tensor.matmul(
                    out=pst[:, :d_out],
                    lhsT=aggT[:k_rows, t * P:(t + 1) * P],
                    rhs=w_sb[:k_rows, :d_out],
                    start=True,
                    stop=(li < 3),
                )
                if li == 3:
                    nc.tensor.matmul(
                        out=pst[:, :d_out],
                        lhsT=ones1_sb[:1, :P],
                        rhs=b4b_sb[:1, :256],
                        start=False,
                        stop=True,
                    )
                    nc.scalar.activation(
                        out=h4nm[:, t * 256:(t + 1) * 256],
                        in_=pst[:, :256], func=AF.Relu)
                else:
                    hb = work.tile([P, 256], bf16, tag="hbuf", bufs=3,
                                   name=f"hbuf_{li}_{t}")
                    nc.scalar.activation(
                        out=hb[:, :d_out], in_=pst[:, :d_out], func=AF.Relu)
                    nc.sync.dma_start(
                        out=tin[li][t * P:(t + 1) * P, :], in_=hb[:, :d_out])

        # ---- layer 1 (sources preloaded on host)
        aggT = scatter_layer(0, 8, lambda ch: xsrc_sb[:, ch * 8:(ch + 1) * 8], None)
        transform_layer(0, aggT, 33, wp1_sb, 32)
        nc.gpsimd.collective_compute(
            "AllGather", mybir.AluOpType.bypass, replica_groups=groups,
            ins=[tin[0].opt()], outs=[tg[0].opt()])

        # ---- layers 2..4
        for li, (d_in, d_out, w_sb) in enumerate(
                [(32, 64, wp2_sb), (64, 128, wp3_sb), (128, 256, w4_sb)], start=1):
            lhsT_of_chunk, prefetch = gathered_layer(li, d_in, tg[li - 1])
            aggT = scatter_layer(li, d_in, lhsT_of_chunk, prefetch)
            k_rows = d_in + 1 if li < 3 else d_in
            transform_layer(li, aggT, k_rows, w_sb, d_out)
            if li < 3:
                nc.gpsimd.collective_compute(
                    "AllGather", mybir.AluOpType.bypass, replica_groups=groups,
                    ins=[tin[li].opt()], outs=[tg[li].opt()])

        # ---- pooling: pooled.T[256, 64] = sum_t h4nm_sub.T @ ghot_sub
        psAB = pp.tile([P, 2 * G], f32, tag="psAB", name="psAB")
        poolsb = work.tile([P, 2 * G], f32, tag="poolsb", name="poolsb")
        for h in range(2):  # one accumulation group per psum bank at a time
            pst = psAB[:, h * G:(h + 1) * G]
            for t in range(NSUB):
                nc.tensor.matmul(
                    out=pst[:, :G],
                    lhsT=h4nm[:, t * 256 + h * P:t * 256 + h * P + P],
                    rhs=ghot_sb[:, t * G:(t + 1) * G],
                    start=(t == 0),
                    stop=(t == NSUB - 1),
                )
            nc.scalar.copy(out=poolsb[:, h * G:(h + 1) * G], in_=pst[:, :G])
        nc.sync.dma_start(out=plin[0:128, :], in_=poolsb[:, 0:G])
        nc.sync.dma_start(out=plin[128:256, :], in_=poolsb[:, G:2 * G])
        nc.gpsimd.collective_compute(
            "AllReduce", mybir.AluOpType.add, replica_groups=groups,
            ins=[plin.opt()], outs=[plg.opt()])

        # ---- head (replicated on every core)
        pool2 = work.tile([P, 2, G], f32, tag="pool2", name="pool2")
        nc.sync.dma_start(out=pool2[:, 0, :], in_=plg[0:128, :])
        nc.sync.dma_start(out=pool2[:, 1, :], in_=plg[128:256, :])
        psHead = pp.tile([P, 512], f32, tag="psHead", name="psHead")
        psc = psHead[:, 128:128 + G]
        nc.tensor.matmul(out=psc[:, :G], lhsT=ones1f_sb[:1, :P],
                         rhs=cinv_sb[:1, :G], start=True, stop=True)
        for h in range(2):
            nc.vector.tensor_tensor(
                out=pool2[:, h, :], in0=pool2[:, h, :],
                in1=psc[:, :G], op=OP.mult)
        psh = psHead[:, 0:G]
        nc.tensor.matmul(out=psh[:NCLS, :G], lhsT=fcw_sb[:, 0, :],
                         rhs=pool2[:, 0, :], start=True, stop=False)
        nc.tensor.matmul(out=psh[:NCLS, :G], lhsT=fcw_sb[:, 1, :],
                         rhs=pool2[:, 1, :], start=False, stop=True)
        lt = work.tile([NCLS, G], f32, tag="lt", name="lt")
        nc.scalar.activation(out=lt[:NCLS, :G], in_=psh[:NCLS, :G],
                             func=AF.Identity, bias=fcb_sb[:NCLS, :1], scale=1.0)
        pstr = psHead[:, 192:192 + NCLS]
        nc.tensor.transpose(out=pstr[:G, :NCLS], in_=lt[:NCLS, :G],
                            identity=ident_sb[:NCLS, :NCLS])
        l2 = work.tile([G, NCLS], f32, tag="l2", name="l2")
        nc.scalar.copy(out=l2[:, :], in_=pstr[:G, :NCLS])

        mx = work.tile([G, 1], f32, tag="mx", name="mx")
        nc.vector.tensor_reduce(out=mx[:, :], in_=l2[:, :],
                                axis=mybir.AxisListType.X, op=OP.max)
        l2m = work.tile([G, NCLS], f32, tag="l2m", name="l2m")
        nc.vector.tensor_scalar_sub(out=l2m[:, :], in0=l2[:, :], scalar1=mx[:, :1])
        ex = work.tile([G, NCLS], f32, tag="ex", name="ex")
        nc.scalar.activation(out=ex[:, :], in_=l2m[:, :], func=AF.Exp)
        sm = work.tile([G, 1], f32, tag="sm", name="sm")
        nc.vector.tensor_reduce(out=sm[:, :], in_=ex[:, :],
                                axis=mybir.AxisListType.X, op=OP.add)
        lsm = work.tile([G, 1], f32, tag="lsm", name="lsm")
        nc.scalar.activation(out=lsm[:, :], in_=sm[:, :], func=AF.Ln)
        res = work.tile([G, NCLS], f32, tag="res", name="res")
        nc.vector.tensor_scalar_sub(out=res[:, :], in0=l2m[:, :], scalar1=lsm[:, :1])
        nc.sync.dma_start(out=out_d[:], in_=res[:, :])

    if finalize:
        nc.finalize()
    return nc


# ------------------------------------------------------------------- entry


def kernel(**inputs):
    from concourse.bass_utils import run_bass_kernel_spmd

    meta, in_maps = build_in_maps(inputs)
    nc = build_program(meta)
    r = run_bass_kernel_spmd(nc, in_maps, list(range(C)))
    return np.asarray(r.results[0]["out"], dtype=np.float32)


if __name__ == "__main__":
    rng = np.random.default_rng(0)
    demo = {
        "x": rng.standard_normal((N, 5), dtype=np.float32),
        "edge_index": rng.integers(0, N, (2, E)).astype(np.int64),
        "batch": np.sort(rng.integers(0, G, N)).astype(np.int64),
        "W1": rng.standard_normal((5, 32), dtype=np.float32) * 0.1,
        "b1": np.zeros(32, np.float32),
        "W2": rng.standard_normal((32, 64), dtype=np.float32) * 0.1,
        "b2": np.zeros(64, np.float32),
        "W3": rng.standard_normal((64, 128), dtype=np.float32) * 0.1,
        "b3": np.zeros(128, np.float32),
        "W4": rng.standard_normal((128, 256), dtype=np.float32) * 0.1,
        "b4": np.zeros(256, np.float32),
        "fcW": rng.standard_normal((256, 10), dtype=np.float32) * 0.1,
        "fcb": np.zeros(10, np.float32),
    }
    print(kernel(**demo))

